# revision 15
# baseline (speedup 1.0000x reference)
"""Causal self-attention (B=4, T=2048, E=768, H=12, D=64) on 8 TRN2 NeuronCores.

Sharding: core c handles batch b = c//2 and head-group g = c%2 (6 heads each).
Per core:
    qT, kT = (x @ WqT + bq).T, ...        stored [384, 2048] (heads x 64, T)
    v      = x @ WvT + bv                 stored fp8 in strip-PAIR tiles
                                          vtp[sp] = [128, 6, 2, 65] (ones col)
    scores per head-pair, per key strip (128 keys), per 512-col piece:
        kT_h[:, s]·qT_h[:, t] for BOTH heads into ONE psum [128, 1024]
        (4 matmuls in 64x64 tiling mode -> quadrants, run concurrently).
        ONE exp op covers both heads' halves, writing FP8 into a strip-pair
        interleaved buffer exb: piece blocks [h0_even, h0_odd, h1_even,
        h1_odd], each pw wide, on a 512 grid based at g=256*sp.
        Odd strips' first 128 cols (queries below their keys) are zeroed once
        at startup; exp writes only the causal-valid cols.
    PV: per head, per t-chunk, per strip-pair sp: ONE fp8 DoubleRow matmul
        (K=256 = both strips) streaming exb blocks [even|odd] as the DR j
        pair, lhsT = vtp[sp][:, h] ([128, 2, 65] with a ones col -> row 64 of
        psum = softmax denominator). 2x fewer streamed columns and 2x rate
        (fp8 DR = 0.5 cycles/row) vs the bf16 M=65 version.
Host: output[b, :, h*64:(h+1)*64] = (outT_h[:64] / outT_h[64:65]).T

q/k projections run entirely in fp8 DoubleRow (weights x64 to avoid
subnormals); v projection stays bf16 (accuracy) and only its OUTPUT is
stored fp8. All matmul writes start on a PSUM bank boundary. Inputs are
host-packed into the exact SBUF layouts.
"""

import numpy as np
import ml_dtypes

import concourse.bacc as bacc
import concourse.mybir as mybir
import concourse.tile as tile
from concourse import bass_utils

F32 = mybir.dt.float32
BF16 = mybir.dt.bfloat16
FP8 = mybir.dt.float8e4
DR = mybir.MatmulPerfMode.DoubleRow
W8SCALE = 64.0      # q/k weights stored x64 in fp8 (else subnormal)

B, T, E, H, D = 4, 2048, 768, 12, 64
NCORES = 8
HPC = 6             # heads per core
OC = HPC * D        # 384 output channels per core
ECH = E // 128      # 6 contraction chunks
QKC = OC // 128     # 3 qT/kT partition chunks (= head pairs)
NSB = T // 128      # 16 key strips
NSP = NSB // 2      # 8 strip pairs
SCALE = 0.125       # 1/sqrt(D)
TCH = 512           # PV t-chunk width
XSL = ECH * 512     # xT free elems per t-slice (3072)

SPW = [T - 256 * sp for sp in range(NSP)]     # pair-grid stored width


def _sp_pieces(sp):
    """(rp, pw) pieces of pair-grid sp on its 512 grid (base g=256*sp)."""
    return [(rp, min(512, SPW[sp] - rp)) for rp in range(0, SPW[sp], 512)]


# exb layout: pair sp at SP_OFF[sp]; piece (rp, pw) at SP_OFF[sp] + 4*rp,
# blocks [h0_even, h0_odd, h1_even, h1_odd] each pw wide.
SP_OFF = [0] * (NSP + 1)
for _sp in range(NSP):
    SP_OFF[_sp + 1] = SP_OFF[_sp] + 4 * SPW[_sp]
EXBW = SP_OFF[NSP]  # 36864


def _strip_pieces(sb):
    """Pieces of strip sb: (rp, pw, qb, qw): piece grid slot (rp, pw) and the
    causal-valid query range [g+qb, g+qb+qw) stored at block cols
    [qb-rp, qb-rp+qw)."""
    sp, par = divmod(sb, 2)
    out = []
    for i, (rp, pw) in enumerate(_sp_pieces(sp)):
        qb, qw = (rp + 128, pw - 128) if (par and i == 0) else (rp, pw)
        out.append((rp, pw, qb, qw))
    return out


def _xoff(e, t):
    """Free-dim offset of (e-chunk, t) in the slice-major packed xT tile.
    Valid for ranges within one 512-wide t slice."""
    return (t // 512) * XSL + 512 * e + (t % 512)


def _build():
    nc = bacc.Bacc("TRN2", debug=False)

    xT_d = nc.dram_tensor("xTp", [128, ECH * T], BF16, kind="ExternalInput")
    xT8_d = nc.dram_tensor("xT8p", [128, ECH * T], FP8, kind="ExternalInput")
    wq_d = nc.dram_tensor("wqp", [128, ECH * OC], FP8, kind="ExternalInput")
    wk_d = nc.dram_tensor("wkp", [128, ECH * OC], FP8, kind="ExternalInput")
    wq16_d = nc.dram_tensor("wq16p", [128, ECH * OC], BF16, kind="ExternalInput")
    wk16_d = nc.dram_tensor("wk16p", [128, ECH * OC], BF16, kind="ExternalInput")
    wv_d = nc.dram_tensor("wvp", [128, ECH * OC], BF16, kind="ExternalInput")
    wv8_d = nc.dram_tensor("wv8p", [128, ECH * OC], FP8, kind="ExternalInput")
    bq_d = nc.dram_tensor("bq", [QKC, 128, 1], F32, kind="ExternalInput")
    bk_d = nc.dram_tensor("bk", [QKC, 128, 1], F32, kind="ExternalInput")
    bvr_d = nc.dram_tensor("bvr", [128, OC], F32, kind="ExternalInput")
    mb_d = nc.dram_tensor("mb", [128, NSB], F32, kind="ExternalInput")
    tri_d = nc.dram_tensor("tri", [128, 128], BF16, kind="ExternalInput")
    out_d = nc.dram_tensor("outT", [HPC, D + 1, T], BF16, kind="ExternalOutput")

    with tile.TileContext(nc) as tc:
        with (
            tc.tile_pool(name="persist", bufs=1) as pp,
            tc.tile_pool(name="qk_ps", bufs=3, space="PSUM") as qk_ps,
            tc.tile_pool(name="b1_ps", bufs=2, space="PSUM") as b1_ps,
            tc.tile_pool(name="stage", bufs=4) as stage,
        ):
            # ---- persistent SBUF tensors ----
            xt_all = pp.tile([128, ECH * T], BF16, tag="xt", name="xt")
            xt8_all = pp.tile([128, ECH * T], FP8, tag="xt8", name="xt8")
            wq_all = pp.tile([128, ECH * OC], FP8, tag="wq", name="wq")
            wk_all = pp.tile([128, ECH * OC], FP8, tag="wk", name="wk")
            wq16_all = pp.tile([128, ECH * OC], BF16, tag="wq16", name="wq16")
            wk16_all = pp.tile([128, ECH * OC], BF16, tag="wk16", name="wk16")
            wv_all = pp.tile([128, ECH * OC], BF16, tag="wv", name="wv")
            wv8_all = pp.tile([128, ECH * OC], FP8, tag="wv8", name="wv8")
            # wq/wk are packed chunk-major: lhsT for (c, e) at cols 768c+128e;
            # for DoubleRow the e-pair ep covers cols 768c+256ep..+256
            wv = [wv_all[:, OC * e:OC * (e + 1)] for e in range(ECH)]

            def wqk_dr(w_all, c, ep):
                o = 768 * c + 256 * ep
                return w_all[:, o:o + 256].rearrange("p (j m) -> p j m", j=2)
            qt = [pp.tile([128, T], BF16, tag=f"qt{c}", name=f"qt{c}") for c in range(QKC)]
            kt = [pp.tile([128, T], BF16, tag=f"kt{c}", name=f"kt{c}") for c in range(QKC)]
            # m-dim padded 65->80: dual-fp8 LDWEIGHTS requires the
            # outer (j) AP step to be a multiple of 16 elements
            vtp = [pp.tile([128, HPC, 2, 80], FP8, tag=f"vt{s}", name=f"vt{s}")
                   for s in range(NSP)]
            exb = pp.tile([128, EXBW], FP8, tag="exb", name="exb")
            # bf16 shadow of sp0's probs + v for queries t < 256: with few
            # keys averaged, fp8 quantization of v/probs hits the output
            # directly (~6%), so the first 256 columns run the PV in bf16
            exb16 = pp.tile([128, 4 * 256], BF16, tag="exb16", name="exb16")
            vt16 = pp.tile([128, HPC, 2, D + 1], BF16, tag="vt16", name="vt16")
            bq_t = [pp.tile([128, 1], F32, tag=f"bq{c}", name=f"bq{c}") for c in range(QKC)]
            bk_t = [pp.tile([128, 1], F32, tag=f"bk{c}", name=f"bk{c}") for c in range(QKC)]
            bvr_t = pp.tile([128, OC], F32, tag="bvr", name="bvr")
            mb_t = pp.tile([128, NSB], F32, tag="mb", name="mb")
            tri_t = pp.tile([128, 128], BF16, tag="tri", name="tri")

            def exb_view(sp, rp, pw):
                X = SP_OFF[sp] + 4 * rp
                return exb[:, X:X + 4 * pw].rearrange(
                    "p (j b w) -> p j b w", j=2, b=2)

            def exb16_view():
                return exb16[:, :].rearrange("p (j b w) -> p j b w", j=2, b=2)

            # ---- input DMAs: contiguous 2D transfers spread over three
            # hardware queues (sync / vector / gpsimd), critical path first.
            # The scalar queue stays DMA-free: it runs the exp chain.
            # Critical chain: xt8 slice 0 + wk/wq c0 -> first k/q projections
            # -> first scores piece -> first exp. ----
            def dma_w(eng, w_all, w_d, c):
                eng.dma_start(w_all[:, 768 * c:768 * c + 768],
                              w_d.ap()[:, 768 * c:768 * c + 768])

            def dma_x(eng, t0):
                s0 = (t0 // 512) * XSL
                eng.dma_start(xt_all[:, s0:s0 + XSL], xT_d.ap()[:, s0:s0 + XSL])

            def dma_x8(eng, t0):
                s0 = (t0 // 512) * XSL
                eng.dma_start(xt8_all[:, s0:s0 + XSL], xT8_d.ap()[:, s0:s0 + XSL])

            # Ring plan. The scalar ring gets EXACTLY 4 DMAs: each ring
            # holds 4 in-flight descriptors, and a 5th trigger BLOCKS the
            # issuing engine's queue -- on scalar that would stall the exp
            # chain behind it. xt slices 1-3 are never loaded (v projection
            # for keys >= 512 runs fp8 DoubleRow from xt8).
            dma_w(nc.scalar, wk16_all, wk16_d, 0)
            dma_w(nc.scalar, wq16_all, wq16_d, 0)
            dma_w(nc.scalar, wk_all, wk_d, 0)
            dma_w(nc.scalar, wq_all, wq_d, 0)
            nc.scalar.dma_start(bvr_t[:, :], bvr_d.ap()[:, :])
            nc.sync.dma_start(xt_all[:, 0:1024], xT_d.ap()[:, 0:1024])
            nc.gpsimd.dma_start(xt_all[:, 1024:2048], xT_d.ap()[:, 1024:2048])
            nc.sync.dma_start(bq_t[0][:, :], bq_d.ap()[0])
            nc.sync.dma_start(bk_t[0][:, :], bk_d.ap()[0])
            nc.gpsimd.dma_start(mb_t[:, :], mb_d.ap()[:, :])
            nc.sync.dma_start(xt_all[:, 2048:2560], xT_d.ap()[:, 2048:2560])
            nc.gpsimd.dma_start(xt_all[:, 2560:XSL], xT_d.ap()[:, 2560:XSL])
            nc.sync.dma_start(xt8_all[:, XSL:XSL + 1536],
                              xT8_d.ap()[:, XSL:XSL + 1536])
            nc.gpsimd.dma_start(xt8_all[:, XSL + 1536:2 * XSL],
                                xT8_d.ap()[:, XSL + 1536:2 * XSL])
            nc.gpsimd.dma_start(tri_t[:, :], tri_d.ap()[:, :])
            nc.sync.dma_start(xt8_all[:, 2 * XSL:2 * XSL + 1536],
                              xT8_d.ap()[:, 2 * XSL:2 * XSL + 1536])
            nc.gpsimd.dma_start(xt8_all[:, 2 * XSL + 1536:3 * XSL],
                                xT8_d.ap()[:, 2 * XSL + 1536:3 * XSL])
            nc.sync.dma_start(wv_all[:, 0:1152], wv_d.ap()[:, 0:1152])
            nc.gpsimd.dma_start(wv_all[:, 1152:2304], wv_d.ap()[:, 1152:2304])
            nc.sync.dma_start(xt8_all[:, 3 * XSL:3 * XSL + 1536],
                              xT8_d.ap()[:, 3 * XSL:3 * XSL + 1536])
            nc.gpsimd.dma_start(xt8_all[:, 3 * XSL + 1536:4 * XSL],
                                xT8_d.ap()[:, 3 * XSL + 1536:4 * XSL])
            nc.gpsimd.dma_start(wv8_all[:, :], wv8_d.ap()[:, :])
            dma_w(nc.sync, wk16_all, wk16_d, 1)
            dma_w(nc.gpsimd, wq16_all, wq16_d, 1)
            dma_w(nc.gpsimd, wk_all, wk_d, 1)
            dma_w(nc.sync, wq_all, wq_d, 1)
            for c in range(1, QKC):
                nc.sync.dma_start(bq_t[c][:, :], bq_d.ap()[c])
                nc.sync.dma_start(bk_t[c][:, :], bk_d.ap()[c])
            dma_w(nc.sync, wk16_all, wk16_d, 2)
            dma_w(nc.gpsimd, wq16_all, wq16_d, 2)
            dma_w(nc.sync, wk_all, wk_d, 2)
            dma_w(nc.gpsimd, wq_all, wq_d, 2)

            # ---- PE warmup: dummy matmuls on a zeroed tile while input DMAs
            # are in flight, so the first projection chain runs at full
            # p-state instead of the 0.65/1.2 GHz cold clock ----
            wu_src = pp.tile([128, 640], BF16, tag="wu", name="wu")
            nc.vector.memset(wu_src[:, :], 0.0)
            wu_ps = b1_ps.tile([128, 512], F32, tag="b1", name="wups")
            for _ in range(16):
                nc.tensor.matmul(wu_ps[:, :], wu_src[:, 0:128],
                                 wu_src[:, 128:640], start=True, stop=True)

            # one-time zeros: odd strips' first 128 block cols (queries below
            # the strip's keys) so PV DoubleRow can stream them blindly
            for sp in range(NSP):
                pw0 = _sp_pieces(sp)[0][1]
                nc.vector.memset(exb_view(sp, 0, pw0)[:, :, 1, 0:128], 0.0)
            nc.vector.memset(exb16_view()[:, :, 1, 0:128], 0.0)

            def proj_qk_chain(c, t0, which):
                # one 512-col chain of the qT or kT projection for chunk c.
                # t0=0 runs bf16 (small-t output rows average few keys, so
                # q/k error there shows directly); t0>=512 rows average many
                # keys and tolerate fp8 DoubleRow.
                ps = b1_ps.tile([128, 512], F32, tag="b1", name="pp")
                if t0 == 0:
                    w16, dst, bias = ((wk16_all, kt, bk_t),
                                      (wq16_all, qt, bq_t))[which]
                    for e in range(ECH):
                        o = 768 * c + 128 * e
                        nc.tensor.matmul(
                            ps[:, :],
                            w16[:, o:o + 128],
                            xt_all[:, _xoff(e, t0):_xoff(e, t0) + 512],
                            start=(e == 0), stop=(e == ECH - 1),
                        )
                    nc.vector.tensor_scalar_add(
                        dst[c][:, t0:t0 + 512], ps[:, :], bias[c][:, 0:1])
                    return
                w_all, dst, bias = ((wk_all, kt, bk_t), (wq_all, qt, bq_t))[which]
                for ep in range(ECH // 2):
                    o = _xoff(2 * ep, t0)
                    nc.tensor.matmul(
                        ps[:, :],
                        wqk_dr(w_all, c, ep),
                        xt8_all[:, o:o + 1024].rearrange(
                            "p (j w) -> p j w", j=2),
                        start=(ep == 0), stop=(ep == ECH // 2 - 1),
                        perf_mode=DR,
                    )
                nc.vector.tensor_scalar(
                    dst[c][:, t0:t0 + 512], ps[:, :], 1.0 / W8SCALE,
                    bias[c][:, 0:1], op0=mybir.AluOpType.mult,
                    op1=mybir.AluOpType.add)

            def proj_v_chain(tb):
                # keys < 512 (tb 0-3) run bf16 from xt slice 0 (small-t
                # accuracy + no extra DMA); keys >= 512 run fp8 DoubleRow
                # from xt8 so xt slices 1-3 never need to be loaded.
                ps = b1_ps.tile([128, OC], F32, tag="b1", name="ppv")
                if tb < 4:
                    for e in range(ECH):
                        o = _xoff(e, 128 * tb)
                        nc.tensor.matmul(
                            ps[:, :],
                            xt_all[:, o:o + 128],
                            wv[e][:, :],
                            start=(e == 0), stop=(e == ECH - 1),
                        )
                else:
                    x8v = xt8_all[:, :].rearrange("p (a w) -> p a w", w=512)
                    t = 128 * tb
                    for ep in range(ECH // 2):
                        a0 = (t // 512) * ECH + 2 * ep
                        c = t % 512
                        nc.tensor.matmul(
                            ps[:, :],
                            x8v[:, a0:a0 + 2, c:c + 128],
                            wv8_all[:, 768 * ep:768 * ep + 768].rearrange(
                                "p (j m) -> p j m", j=2),
                            start=(ep == 0), stop=(ep == ECH // 2 - 1),
                            perf_mode=DR,
                        )
                sp, jj = divmod(tb, 2)
                nc.vector.memset(vtp[sp][:, :, jj, D:D + 1], 1.0)
                if tb < 4:
                    nc.vector.tensor_tensor(
                        vtp[sp][:, :, jj, 0:D],
                        ps.rearrange("p (h d) -> p h d", h=HPC),
                        bvr_t.rearrange("p (h d) -> p h d", h=HPC),
                        op=mybir.AluOpType.add,
                    )
                else:
                    nc.vector.scalar_tensor_tensor(
                        vtp[sp][:, :, jj, 0:D],
                        ps.rearrange("p (h d) -> p h d", h=HPC),
                        1.0 / W8SCALE,
                        bvr_t.rearrange("p (h d) -> p h d", h=HPC),
                        op0=mybir.AluOpType.mult,
                        op1=mybir.AluOpType.add,
                    )
                if sp == 0:
                    nc.vector.memset(vt16[:, :, jj, D:D + 1], 1.0)
                    nc.vector.tensor_tensor(
                        vt16[:, :, jj, 0:D],
                        ps.rearrange("p (h d) -> p h d", h=HPC),
                        bvr_t.rearrange("p (h d) -> p h d", h=HPC),
                        op=mybir.AluOpType.add,
                    )

            def qk_exp_piece(h0, sb, p, rp, pw, qb, qw):
                # piece p of strip sb for the pair (h0, h0+1): four adjacent
                # 64x64-mode matmuls into one psum tile (head half x key
                # half quadrants run concurrently), then ONE exp over both
                # heads' halves writing fp8 into the strip-pair layout.
                c = h0 // 2
                sp, par = divmod(sb, 2)
                t0 = 128 * sb
                g = 256 * sp
                ps = qk_ps.tile([128, 1024], F32, tag="qk", name="qk")
                for ofs, rows in ((0, slice(0, 64)), (512, slice(64, 128))):
                    for so, pr in ((0, slice(0, 64)), (64, slice(64, 128))):
                        nc.tensor.matmul(
                            ps[pr, ofs:ofs + qw],
                            kt[c][rows, t0 + so:t0 + so + 64],
                            qt[c][rows, g + qb:g + qb + qw],
                            start=True, stop=True,
                        )
                srcv = ps[:, :].rearrange("p (j w) -> p j w", j=2)
                src = srcv[:, :, 0:qw]
                dst = exb_view(sp, rp, pw)[:, :, par, pw - qw:pw]
                nc.scalar.activation(
                    dst, src,
                    mybir.ActivationFunctionType.Exp,
                    bias=mb_t[:, sb:sb + 1], scale=SCALE)
                if sb <= 1 and p == 0:
                    # bf16 shadow for queries [0, 256): strip0 psum cols
                    # [0,256) -> block cols [0,256); strip1 psum cols [0,128)
                    # (queries [128,256)) -> block cols [128,256)
                    q16 = 256 - 128 * par
                    nc.scalar.activation(
                        exb16_view()[:, :, par, 256 - q16:256],
                        srcv[:, :, 0:q16],
                        mybir.ActivationFunctionType.Exp,
                        bias=mb_t[:, sb:sb + 1], scale=SCALE)

            def tri_strip(h, sb):
                # causal mask on the diagonal 128x128 block of strip sb
                sp, par = divmod(sb, 2)
                pw0 = _sp_pieces(sp)[0][1]
                o = SP_OFF[sp] + (2 * (h % 2) + par) * pw0 + par * 128
                nc.vector.tensor_tensor(
                    exb[:, o:o + 128], exb[:, o:o + 128], tri_t[:, :],
                    op=mybir.AluOpType.mult)
                if sb <= 1:
                    o16 = (2 * (h % 2) + par) * 256 + par * 128
                    nc.vector.tensor_tensor(
                        exb16[:, o16:o16 + 128], exb16[:, o16:o16 + 128],
                        tri_t[:, :], op=mybir.AluOpType.mult)

            def pv_chunk(h, tc0, W=TCH):
                ps = b1_ps.tile([80, W], F32, tag="b1", name="pv",
                                padded_shape=[80, TCH])
                hh = h % 2
                if tc0 + W <= 256:
                    # bf16 path: strips 0 and 1 from the bf16 shadow
                    for par in range(2):
                        nc.tensor.matmul(
                            ps[0:D + 1, :],
                            vt16[:, h, par, :],
                            exb16_view()[:, hh, par, tc0:tc0 + W],
                            start=(par == 0), stop=(par == 1),
                        )
                    st = stage.tile([D + 1, W], BF16, tag="st", name="st",
                                    padded_shape=[D + 1, TCH])
                    nc.vector.tensor_copy(st[:, :], ps[0:D + 1, :])
                    out_eng = nc.sync if (h + tc0 // 128) % 2 == 0 else nc.gpsimd
                    out_eng.dma_start(out_d.ap()[h, :, tc0:tc0 + W], st[:, :])
                    return
                segs = []
                for sp in range(NSP):
                    g = 256 * sp
                    if g >= tc0 + W:
                        break
                    for rp, pw in _sp_pieces(sp):
                        s_lo = max(tc0, g + rp)
                        s_hi = min(tc0 + W, g + rp + pw)
                        if s_lo >= s_hi:
                            continue
                        # a window that never reaches the odd strip's keys
                        # uses a single-row matmul on the even strip only
                        # (the odd v tile may not be written yet)
                        segs.append((sp, rp, pw, s_lo - g - rp,
                                     s_lo - tc0, s_hi - s_lo,
                                     s_hi > g + 128))
                for i, (sp, rp, pw, c0, oc0, w, dr) in enumerate(segs):
                    view = exb_view(sp, rp, pw)
                    nc.tensor.matmul(
                        ps[:, oc0:oc0 + w],
                        vtp[sp][:, h, :, :] if dr else vtp[sp][:, h, 0, :],
                        view[:, hh, :, c0:c0 + w] if dr
                        else view[:, hh, 0, c0:c0 + w],
                        start=(i == 0), stop=(i == len(segs) - 1),
                        perf_mode=DR if dr else None,
                    )
                st = stage.tile([D + 1, W], BF16, tag="st", name="st",
                                padded_shape=[D + 1, TCH])
                nc.vector.tensor_copy(st[:, :], ps[0:D + 1, :])
                out_eng = nc.sync if (h + tc0 // 128) % 2 == 0 else nc.gpsimd
                out_eng.dma_start(out_d.ap()[h, :, tc0:tc0 + W], st[:, :])

            # pv windows: (emit-at-strip, tc0, width). The tail windows are
            # split so most of the last chunk's accumulation runs before the
            # final strips, shrinking the pair-boundary stall.
            PV_WINDOWS = {5: [(0, 256), (256, 256)], 7: [(512, 512)],
                          11: [(1024, 512)], 13: [(1536, 256)],
                          14: [(1792, 128)], 15: [(1920, 128)]}

            # ---- pipelined emission: a global slot pipeline ----
            # Pair p's strips occupy slots [12p, 12p+16); consecutive pairs
            # overlap by 4 slots so ACT never idles at pair boundaries. A
            # piece whose expT region is still to be read by the previous
            # pair's late pv windows (t >= 1536) is deferred past them.
            NSLOT = 41
            slot_work = [[] for _ in range(NSLOT)]   # items: (kind, fn)
            # kind 0: scores pieces/tri; 1: proj fillers; 2: pv windows

            # pair (0,1) cascaded start: emit chunk-0 projection chains
            # t-ascending; after each chain emit every scores piece whose q/k
            # columns are available, so the first exp fires early.
            all_pieces = [(256 * (sb // 2) + qb + qw, sb, p, rp, pw, qb, qw)
                          for sb in range(NSB)
                          for p, (rp, pw, qb, qw) in enumerate(_strip_pieces(sb))]
            all_pieces.sort(key=lambda x: (x[0], x[1]))
            emitted = set()
            tri_done = set()

            def emit_ready(limit):
                for need, sb, p, rp, pw, qb, qw in all_pieces:
                    if need > limit:
                        break
                    if (sb, p) in emitted:
                        continue
                    qk_exp_piece(0, sb, p, rp, pw, qb, qw)
                    emitted.add((sb, p))
                    if p == 0:
                        tri_strip(0, sb)
                        tri_strip(1, sb)
                        tri_done.add(sb)

            for t0 in range(0, T, 512):
                proj_qk_chain(0, t0, 0)
                proj_qk_chain(0, t0, 1)
                emit_ready(min(t0 + 512, 1024))

            def sched_pair(h0, base, defer_slot):
                for sb in range(NSB):
                    for p, (rp, pw, qb, qw) in enumerate(_strip_pieces(sb)):
                        if h0 == 0 and (sb, p) in emitted:
                            continue
                        t_end = 256 * (sb // 2) + qb + qw
                        sl = base + sb
                        if t_end > 1536 and sl < defer_slot:
                            sl = defer_slot
                        work = [(0, lambda h0=h0, sb=sb, p=p, rp=rp, pw=pw,
                                 qb=qb, qw=qw:
                                 qk_exp_piece(h0, sb, p, rp, pw, qb, qw))]
                        if p == 0 and not (h0 == 0 and sb in tri_done):
                            work.append((0, lambda h0=h0, sb=sb: tri_strip(h0, sb)))
                            work.append((0, lambda h0=h0, sb=sb: tri_strip(h0 + 1, sb)))
                        slot_work[sl].extend(work)
                    for tc0, W in PV_WINDOWS.get(sb, ()):
                        sl = max(base + sb, defer_slot if base else 0)
                        slot_work[sl].append(
                            (2, lambda h0=h0, tc0=tc0, W=W: pv_chunk(h0, tc0, W)))
                        slot_work[sl].append(
                            (2, lambda h0=h0, tc0=tc0, W=W: pv_chunk(h0 + 1, tc0, W)))

            sched_pair(0, 0, 0)
            sched_pair(2, 12, 16)
            sched_pair(4, 24, 28)
            # fillers: v projection paced over pair01's strips (1/slot, just
            # ahead of the pv windows); chunk-1/2 q/k projections early
            for tb in range(NSB):
                slot_work[4 + tb // 2 if tb < 8 else tb].append(
                    (1, lambda tb=tb: proj_v_chain(tb)))
            # c1/c2 chains placed as late as their consumers allow, to keep
            # the PE free for pair-0/1 scores and relax the weight-DMA
            # deadlines (c1 needed from slot 12, c2 from slot 24)
            for i in range(8):
                t0, wch = (i // 2) * 512, i % 2
                slot_work[8 + i // 2].append(
                    (1, lambda t0=t0, w=wch: proj_qk_chain(1, t0, w)))
                slot_work[20 + i // 2].append(
                    (1, lambda t0=t0, w=wch: proj_qk_chain(2, t0, w)))

            for sl in range(NSLOT):
                for _, f in sorted(slot_work[sl], key=lambda kf: kf[0]):
                    f()

    nc.compile()
    return nc


_NC_CACHE = None


def _get_nc():
    global _NC_CACHE
    if _NC_CACHE is None:
        _NC_CACHE = _build()
    return _NC_CACHE


def _pack_x(xb, dt=ml_dtypes.bfloat16):
    """[T, E] batch slice -> slice-major packed [128, ECH*T] (xT layout)."""
    xT = xb.T.reshape(ECH, 128, T // 512, 512)          # [e, p, s, t']
    return np.ascontiguousarray(
        xT.transpose(1, 2, 0, 3).reshape(128, ECH * T)).astype(dt)


def _pack_w(w_sl, dt=ml_dtypes.bfloat16, scale=1.0):
    """[384, 768] weight slice -> e-major packed [128, ECH*OC] (for wv:
    rhs slice for e-chunk at cols [OC*e, OC*(e+1)))."""
    wT = (w_sl.T * scale).reshape(ECH, 128, OC)         # [e, p, j]
    return np.ascontiguousarray(
        wT.transpose(1, 0, 2).reshape(128, ECH * OC)).astype(dt)


def _pack_w_cm(w_sl, dt=ml_dtypes.float8_e4m3, scale=W8SCALE):
    """[384, 768] weight slice -> chunk-major packed [128, ECH*OC]:
    lhsT for (chunk c, e-chunk e) at cols [768c+128e, 768c+128e+128)."""
    wT = (w_sl.T * scale).reshape(ECH, 128, QKC, 128)   # [e, p, c, j]
    return np.ascontiguousarray(
        wT.transpose(1, 2, 0, 3).reshape(128, ECH * OC)).astype(dt)


def kernel(hidden_states, attention_mask, Wq, bq, Wk, bk, Wv, bv):
    nc = _get_nc()
    in_maps = _make_in_maps(hidden_states, attention_mask, Wq, bq, Wk, bk, Wv, bv)
    res = bass_utils.run_bass_kernel_spmd(nc, in_maps, core_ids=list(range(NCORES)))
    return _assemble(res.results)


def _make_in_maps(hidden_states, attention_mask, Wq, bq, Wk, bk, Wv, bv):
    hidden_states = np.asarray(hidden_states, dtype=np.float32)
    attention_mask = np.asarray(attention_mask, dtype=np.float32)
    Wq, Wk, Wv = (np.asarray(w, dtype=np.float32) for w in (Wq, Wk, Wv))
    bq, bk, bv = (np.asarray(b, dtype=np.float32) for b in (bq, bk, bv))

    tri = np.tril(np.ones((128, 128), np.float32)).T.astype(ml_dtypes.bfloat16)
    # tri[s, t] = 1 where t >= s

    in_maps = []
    for c in range(NCORES):
        b, g = divmod(c, 2)
        sl = slice(OC * g, OC * (g + 1))
        m = attention_mask[b, 0, 0, :]
        in_maps.append({
            "xTp": _pack_x(hidden_states[b]),
            "xT8p": _pack_x(hidden_states[b], ml_dtypes.float8_e4m3),
            "wqp": _pack_w_cm(Wq[sl]),
            "wkp": _pack_w_cm(Wk[sl]),
            "wq16p": _pack_w_cm(Wq[sl], ml_dtypes.bfloat16, 1.0),
            "wk16p": _pack_w_cm(Wk[sl], ml_dtypes.bfloat16, 1.0),
            "wvp": _pack_w(Wv[sl]),
            "wv8p": _pack_w(Wv[sl], ml_dtypes.float8_e4m3, W8SCALE),
            "bq": np.ascontiguousarray(bq[sl]).reshape(QKC, 128, 1),
            "bk": np.ascontiguousarray(bk[sl]).reshape(QKC, 128, 1),
            "bvr": np.broadcast_to(bv[sl], (128, OC)).copy(),
            "mb": np.ascontiguousarray(m.reshape(NSB, 128).T),
            "tri": tri,
        })
    return in_maps


def _assemble(results):
    out = np.empty((B, T, E), np.float32)
    for c in range(NCORES):
        b, g = divmod(c, 2)
        oT = np.asarray(results[c]["outT"], dtype=np.float32)  # [6, 65, 2048]
        for h6 in range(HPC):
            h = HPC * g + h6
            out[b, :, D * h:D * h + D] = (oT[h6, :D] / oT[h6, D:D + 1]).T
    return out


# revision 16
# speedup vs baseline: 1.0635x; 1.0635x over previous
"""Causal self-attention (B=4, T=2048, E=768, H=12, D=64) on 8 TRN2 NeuronCores.

Sharding: core c handles batch b = c//2 and head-group g = c%2 (6 heads each).
Per core:
    qT, kT = (x @ WqT + bq).T, ...        stored [384, 2048] (heads x 64, T)
    v      = x @ WvT + bv                 stored fp8 in strip-PAIR tiles
                                          vtp[sp] = [128, 6, 2, 65] (ones col)
    scores per head-pair, per key strip (128 keys), per 512-col piece:
        kT_h[:, s]·qT_h[:, t] for BOTH heads into ONE psum [128, 1024]
        (4 matmuls in 64x64 tiling mode -> quadrants, run concurrently).
        ONE exp op covers both heads' halves, writing FP8 into a strip-pair
        interleaved buffer exb: piece blocks [h0_even, h0_odd, h1_even,
        h1_odd], each pw wide, on a 512 grid based at g=256*sp.
        Odd strips' first 128 cols (queries below their keys) are zeroed once
        at startup; exp writes only the causal-valid cols.
    PV: per head, per t-chunk, per strip-pair sp: ONE fp8 DoubleRow matmul
        (K=256 = both strips) streaming exb blocks [even|odd] as the DR j
        pair, lhsT = vtp[sp][:, h] ([128, 2, 65] with a ones col -> row 64 of
        psum = softmax denominator). 2x fewer streamed columns and 2x rate
        (fp8 DR = 0.5 cycles/row) vs the bf16 M=65 version.
Host: output[b, :, h*64:(h+1)*64] = (outT_h[:64] / outT_h[64:65]).T

q/k projections run entirely in fp8 DoubleRow (weights x64 to avoid
subnormals); v projection stays bf16 (accuracy) and only its OUTPUT is
stored fp8. All matmul writes start on a PSUM bank boundary. Inputs are
host-packed into the exact SBUF layouts.
"""

import numpy as np
import ml_dtypes

import concourse.bacc as bacc
import concourse.mybir as mybir
import concourse.tile as tile
from concourse import bass_utils

F32 = mybir.dt.float32
BF16 = mybir.dt.bfloat16
FP8 = mybir.dt.float8e4
DR = mybir.MatmulPerfMode.DoubleRow
W8SCALE = 64.0      # q/k weights stored x64 in fp8 (else subnormal)

B, T, E, H, D = 4, 2048, 768, 12, 64
NCORES = 8
HPC = 6             # heads per core
OC = HPC * D        # 384 output channels per core
ECH = E // 128      # 6 contraction chunks
QKC = OC // 128     # 3 qT/kT partition chunks (= head pairs)
NSB = T // 128      # 16 key strips
NSP = NSB // 2      # 8 strip pairs
SCALE = 0.125       # 1/sqrt(D)
TCH = 512           # PV t-chunk width
XSL = ECH * 512     # xT free elems per t-slice (3072)

SPW = [T - 256 * sp for sp in range(NSP)]     # pair-grid stored width


def _sp_pieces(sp):
    """(rp, pw) pieces of pair-grid sp on its 512 grid (base g=256*sp)."""
    return [(rp, min(512, SPW[sp] - rp)) for rp in range(0, SPW[sp], 512)]


# exb layout: pair sp at SP_OFF[sp]; piece (rp, pw) at SP_OFF[sp] + 4*rp,
# blocks [h0_even, h0_odd, h1_even, h1_odd] each pw wide.
SP_OFF = [0] * (NSP + 1)
for _sp in range(NSP):
    SP_OFF[_sp + 1] = SP_OFF[_sp] + 4 * SPW[_sp]
EXBW = SP_OFF[NSP]  # 36864


def _strip_pieces(sb):
    """Pieces of strip sb: (rp, pw, qb, qw): piece grid slot (rp, pw) and the
    causal-valid query range [g+qb, g+qb+qw) stored at block cols
    [qb-rp, qb-rp+qw)."""
    sp, par = divmod(sb, 2)
    out = []
    for i, (rp, pw) in enumerate(_sp_pieces(sp)):
        qb, qw = (rp + 128, pw - 128) if (par and i == 0) else (rp, pw)
        out.append((rp, pw, qb, qw))
    return out


def _xoff(e, t):
    """Free-dim offset of (e-chunk, t) in the slice-major packed xT tile.
    Valid for ranges within one 512-wide t slice."""
    return (t // 512) * XSL + 512 * e + (t % 512)


def _build():
    nc = bacc.Bacc("TRN2", debug=False)

    xT_d = nc.dram_tensor("xTp", [128, ECH * T], BF16, kind="ExternalInput")
    xT8_d = nc.dram_tensor("xT8p", [128, ECH * T], FP8, kind="ExternalInput")
    wq_d = nc.dram_tensor("wqp", [128, ECH * OC], FP8, kind="ExternalInput")
    wk_d = nc.dram_tensor("wkp", [128, ECH * OC], FP8, kind="ExternalInput")
    wq16_d = nc.dram_tensor("wq16p", [128, ECH * OC], BF16, kind="ExternalInput")
    wk16_d = nc.dram_tensor("wk16p", [128, ECH * OC], BF16, kind="ExternalInput")
    wv_d = nc.dram_tensor("wvp", [128, ECH * OC], BF16, kind="ExternalInput")
    wv8_d = nc.dram_tensor("wv8p", [128, ECH * OC], FP8, kind="ExternalInput")
    bq_d = nc.dram_tensor("bq", [QKC, 128, 1], F32, kind="ExternalInput")
    bk_d = nc.dram_tensor("bk", [QKC, 128, 1], F32, kind="ExternalInput")
    bvr_d = nc.dram_tensor("bvr", [128, OC], F32, kind="ExternalInput")
    mb_d = nc.dram_tensor("mb", [128, NSB], F32, kind="ExternalInput")
    tri_d = nc.dram_tensor("tri", [128, 128], BF16, kind="ExternalInput")
    out_d = nc.dram_tensor("outT", [HPC, D + 1, T], BF16, kind="ExternalOutput")

    with tile.TileContext(nc) as tc:
        with (
            tc.tile_pool(name="persist", bufs=1) as pp,
            tc.tile_pool(name="qk_ps", bufs=3, space="PSUM") as qk_ps,
            tc.tile_pool(name="b1_ps", bufs=2, space="PSUM") as b1_ps,
            tc.tile_pool(name="stage", bufs=4) as stage,
        ):
            # ---- persistent SBUF tensors ----
            xt_all = pp.tile([128, ECH * T], BF16, tag="xt", name="xt")
            xt8_all = pp.tile([128, ECH * T], FP8, tag="xt8", name="xt8")
            wq_all = pp.tile([128, ECH * OC], FP8, tag="wq", name="wq")
            wk_all = pp.tile([128, ECH * OC], FP8, tag="wk", name="wk")
            wq16_all = pp.tile([128, ECH * OC], BF16, tag="wq16", name="wq16")
            wk16_all = pp.tile([128, ECH * OC], BF16, tag="wk16", name="wk16")
            wv_all = pp.tile([128, ECH * OC], BF16, tag="wv", name="wv")
            wv8_all = pp.tile([128, ECH * OC], FP8, tag="wv8", name="wv8")
            # wq/wk are packed chunk-major: lhsT for (c, e) at cols 768c+128e;
            # for DoubleRow the e-pair ep covers cols 768c+256ep..+256
            wv = [wv_all[:, OC * e:OC * (e + 1)] for e in range(ECH)]

            def wqk_dr(w_all, c, ep):
                o = 768 * c + 256 * ep
                return w_all[:, o:o + 256].rearrange("p (j m) -> p j m", j=2)
            qt = [pp.tile([128, T], BF16, tag=f"qt{c}", name=f"qt{c}") for c in range(QKC)]
            kt = [pp.tile([128, T], BF16, tag=f"kt{c}", name=f"kt{c}") for c in range(QKC)]
            # m-dim padded 65->80: dual-fp8 LDWEIGHTS requires the
            # outer (j) AP step to be a multiple of 16 elements
            vtp = [pp.tile([128, HPC, 2, 80], FP8, tag=f"vt{s}", name=f"vt{s}")
                   for s in range(NSP)]
            exb = pp.tile([128, EXBW], FP8, tag="exb", name="exb")
            # bf16 shadow of sp0's probs + v for queries t < 256: with few
            # keys averaged, fp8 quantization of v/probs hits the output
            # directly (~6%), so the first 256 columns run the PV in bf16
            exb16 = pp.tile([128, 4 * 256], BF16, tag="exb16", name="exb16")
            vt16 = pp.tile([128, HPC, 2, D + 1], BF16, tag="vt16", name="vt16")
            bq_t = [pp.tile([128, 1], F32, tag=f"bq{c}", name=f"bq{c}") for c in range(QKC)]
            bk_t = [pp.tile([128, 1], F32, tag=f"bk{c}", name=f"bk{c}") for c in range(QKC)]
            bvr_t = pp.tile([128, OC], F32, tag="bvr", name="bvr")
            mb_t = pp.tile([128, NSB], F32, tag="mb", name="mb")
            tri_t = pp.tile([128, 128], BF16, tag="tri", name="tri")

            def exb_view(sp, rp, pw):
                X = SP_OFF[sp] + 4 * rp
                return exb[:, X:X + 4 * pw].rearrange(
                    "p (j b w) -> p j b w", j=2, b=2)

            def exb16_view():
                return exb16[:, :].rearrange("p (j b w) -> p j b w", j=2, b=2)

            # ---- input DMAs: contiguous 2D transfers spread over three
            # hardware queues (sync / vector / gpsimd), critical path first.
            # The scalar queue stays DMA-free: it runs the exp chain.
            # Critical chain: xt8 slice 0 + wk/wq c0 -> first k/q projections
            # -> first scores piece -> first exp. ----
            def dma_w(eng, w_all, w_d, c):
                eng.dma_start(w_all[:, 768 * c:768 * c + 768],
                              w_d.ap()[:, 768 * c:768 * c + 768])

            def dma_x(eng, t0):
                s0 = (t0 // 512) * XSL
                eng.dma_start(xt_all[:, s0:s0 + XSL], xT_d.ap()[:, s0:s0 + XSL])

            def dma_x8(eng, t0):
                s0 = (t0 // 512) * XSL
                eng.dma_start(xt8_all[:, s0:s0 + XSL], xT8_d.ap()[:, s0:s0 + XSL])

            # Ring plan. The scalar ring gets EXACTLY 4 DMAs: each ring
            # holds 4 in-flight descriptors, and a 5th trigger BLOCKS the
            # issuing engine's queue -- on scalar that would stall the exp
            # chain behind it. xt slices 1-3 are never loaded (v projection
            # for keys >= 512 runs fp8 DoubleRow from xt8).
            dma_w(nc.scalar, wk16_all, wk16_d, 0)
            dma_w(nc.scalar, wq16_all, wq16_d, 0)
            dma_w(nc.scalar, wk_all, wk_d, 0)
            dma_w(nc.scalar, wq_all, wq_d, 0)
            nc.scalar.dma_start(bvr_t[:, :], bvr_d.ap()[:, :])
            nc.sync.dma_start(xt_all[:, 0:1024], xT_d.ap()[:, 0:1024])
            nc.gpsimd.dma_start(xt_all[:, 1024:2048], xT_d.ap()[:, 1024:2048])
            nc.sync.dma_start(bq_t[0][:, :], bq_d.ap()[0])
            nc.sync.dma_start(bk_t[0][:, :], bk_d.ap()[0])
            nc.gpsimd.dma_start(mb_t[:, :], mb_d.ap()[:, :])
            nc.sync.dma_start(xt_all[:, 2048:2560], xT_d.ap()[:, 2048:2560])
            nc.gpsimd.dma_start(xt_all[:, 2560:XSL], xT_d.ap()[:, 2560:XSL])
            nc.sync.dma_start(xt8_all[:, XSL:XSL + 1536],
                              xT8_d.ap()[:, XSL:XSL + 1536])
            nc.gpsimd.dma_start(xt8_all[:, XSL + 1536:2 * XSL],
                                xT8_d.ap()[:, XSL + 1536:2 * XSL])
            nc.gpsimd.dma_start(tri_t[:, :], tri_d.ap()[:, :])
            nc.sync.dma_start(xt8_all[:, 2 * XSL:2 * XSL + 1536],
                              xT8_d.ap()[:, 2 * XSL:2 * XSL + 1536])
            nc.gpsimd.dma_start(xt8_all[:, 2 * XSL + 1536:3 * XSL],
                                xT8_d.ap()[:, 2 * XSL + 1536:3 * XSL])
            nc.sync.dma_start(wv_all[:, 0:1152], wv_d.ap()[:, 0:1152])
            nc.gpsimd.dma_start(wv_all[:, 1152:2304], wv_d.ap()[:, 1152:2304])
            nc.sync.dma_start(xt8_all[:, 3 * XSL:3 * XSL + 1536],
                              xT8_d.ap()[:, 3 * XSL:3 * XSL + 1536])
            nc.gpsimd.dma_start(xt8_all[:, 3 * XSL + 1536:4 * XSL],
                                xT8_d.ap()[:, 3 * XSL + 1536:4 * XSL])
            nc.gpsimd.dma_start(wv8_all[:, :], wv8_d.ap()[:, :])
            dma_w(nc.sync, wk16_all, wk16_d, 1)
            dma_w(nc.gpsimd, wq16_all, wq16_d, 1)
            dma_w(nc.gpsimd, wk_all, wk_d, 1)
            dma_w(nc.sync, wq_all, wq_d, 1)
            for c in range(1, QKC):
                nc.sync.dma_start(bq_t[c][:, :], bq_d.ap()[c])
                nc.sync.dma_start(bk_t[c][:, :], bk_d.ap()[c])
            dma_w(nc.sync, wk16_all, wk16_d, 2)
            dma_w(nc.gpsimd, wq16_all, wq16_d, 2)
            dma_w(nc.sync, wk_all, wk_d, 2)
            dma_w(nc.gpsimd, wq_all, wq_d, 2)

            # ---- PE warmup: dummy matmuls on a zeroed tile while input DMAs
            # are in flight, so the first projection chain runs at full
            # p-state instead of the 0.65/1.2 GHz cold clock ----
            wu_src = pp.tile([128, 640], BF16, tag="wu", name="wu")
            nc.vector.memset(wu_src[:, :], 0.0)
            wu_ps = b1_ps.tile([128, 512], F32, tag="b1", name="wups")
            for _ in range(16):
                nc.tensor.matmul(wu_ps[:, :], wu_src[:, 0:128],
                                 wu_src[:, 128:640], start=True, stop=True)

            # one-time zeros: odd strips' first 128 block cols (queries below
            # the strip's keys) so PV DoubleRow can stream them blindly
            for sp in range(NSP):
                pw0 = _sp_pieces(sp)[0][1]
                nc.vector.memset(exb_view(sp, 0, pw0)[:, :, 1, 0:128], 0.0)
            nc.vector.memset(exb16_view()[:, :, 1, 0:128], 0.0)

            def proj_qk_chain(c, t0, which):
                # one 512-col chain of the qT or kT projection for chunk c.
                # t0=0 runs bf16 (small-t output rows average few keys, so
                # q/k error there shows directly); t0>=512 rows average many
                # keys and tolerate fp8 DoubleRow.
                ps = b1_ps.tile([128, 512], F32, tag="b1", name="pp")
                if t0 == 0:
                    w16, dst, bias = ((wk16_all, kt, bk_t),
                                      (wq16_all, qt, bq_t))[which]
                    for e in range(ECH):
                        o = 768 * c + 128 * e
                        nc.tensor.matmul(
                            ps[:, :],
                            w16[:, o:o + 128],
                            xt_all[:, _xoff(e, t0):_xoff(e, t0) + 512],
                            start=(e == 0), stop=(e == ECH - 1),
                        )
                    nc.vector.tensor_scalar_add(
                        dst[c][:, t0:t0 + 512], ps[:, :], bias[c][:, 0:1])
                    return
                w_all, dst, bias = ((wk_all, kt, bk_t), (wq_all, qt, bq_t))[which]
                for ep in range(ECH // 2):
                    o = _xoff(2 * ep, t0)
                    nc.tensor.matmul(
                        ps[:, :],
                        wqk_dr(w_all, c, ep),
                        xt8_all[:, o:o + 1024].rearrange(
                            "p (j w) -> p j w", j=2),
                        start=(ep == 0), stop=(ep == ECH // 2 - 1),
                        perf_mode=DR,
                    )
                nc.vector.tensor_scalar(
                    dst[c][:, t0:t0 + 512], ps[:, :], 1.0 / W8SCALE,
                    bias[c][:, 0:1], op0=mybir.AluOpType.mult,
                    op1=mybir.AluOpType.add)

            def proj_v_chain(tb):
                # keys < 512 (tb 0-3) run bf16 from xt slice 0 (small-t
                # accuracy + no extra DMA); keys >= 512 run fp8 DoubleRow
                # from xt8 so xt slices 1-3 never need to be loaded.
                ps = b1_ps.tile([128, OC], F32, tag="b1", name="ppv")
                if tb < 4:
                    for e in range(ECH):
                        o = _xoff(e, 128 * tb)
                        nc.tensor.matmul(
                            ps[:, :],
                            xt_all[:, o:o + 128],
                            wv[e][:, :],
                            start=(e == 0), stop=(e == ECH - 1),
                        )
                else:
                    x8v = xt8_all[:, :].rearrange("p (a w) -> p a w", w=512)
                    t = 128 * tb
                    for ep in range(ECH // 2):
                        a0 = (t // 512) * ECH + 2 * ep
                        c = t % 512
                        nc.tensor.matmul(
                            ps[:, :],
                            x8v[:, a0:a0 + 2, c:c + 128],
                            wv8_all[:, 768 * ep:768 * ep + 768].rearrange(
                                "p (j m) -> p j m", j=2),
                            start=(ep == 0), stop=(ep == ECH // 2 - 1),
                            perf_mode=DR,
                        )
                sp, jj = divmod(tb, 2)
                nc.vector.memset(vtp[sp][:, :, jj, D:D + 1], 1.0)
                if tb < 4:
                    nc.vector.tensor_tensor(
                        vtp[sp][:, :, jj, 0:D],
                        ps.rearrange("p (h d) -> p h d", h=HPC),
                        bvr_t.rearrange("p (h d) -> p h d", h=HPC),
                        op=mybir.AluOpType.add,
                    )
                else:
                    nc.vector.scalar_tensor_tensor(
                        vtp[sp][:, :, jj, 0:D],
                        ps.rearrange("p (h d) -> p h d", h=HPC),
                        1.0 / W8SCALE,
                        bvr_t.rearrange("p (h d) -> p h d", h=HPC),
                        op0=mybir.AluOpType.mult,
                        op1=mybir.AluOpType.add,
                    )
                if sp == 0:
                    nc.vector.memset(vt16[:, :, jj, D:D + 1], 1.0)
                    nc.vector.tensor_tensor(
                        vt16[:, :, jj, 0:D],
                        ps.rearrange("p (h d) -> p h d", h=HPC),
                        bvr_t.rearrange("p (h d) -> p h d", h=HPC),
                        op=mybir.AluOpType.add,
                    )

            def qk_exp_piece(h0, sb, p, rp, pw, qb, qw):
                # piece p of strip sb for the pair (h0, h0+1): four adjacent
                # 64x64-mode matmuls into one psum tile (head half x key
                # half quadrants run concurrently), then ONE exp over both
                # heads' halves writing fp8 into the strip-pair layout.
                c = h0 // 2
                sp, par = divmod(sb, 2)
                t0 = 128 * sb
                g = 256 * sp
                ps = qk_ps.tile([128, 1024], F32, tag="qk", name="qk")
                for ofs, rows in ((0, slice(0, 64)), (512, slice(64, 128))):
                    for so, pr in ((0, slice(0, 64)), (64, slice(64, 128))):
                        nc.tensor.matmul(
                            ps[pr, ofs:ofs + qw],
                            kt[c][rows, t0 + so:t0 + so + 64],
                            qt[c][rows, g + qb:g + qb + qw],
                            start=True, stop=True,
                        )
                srcv = ps[:, :].rearrange("p (j w) -> p j w", j=2)
                src = srcv[:, :, 0:qw]
                dst = exb_view(sp, rp, pw)[:, :, par, pw - qw:pw]
                nc.scalar.activation(
                    dst, src,
                    mybir.ActivationFunctionType.Exp,
                    bias=mb_t[:, sb:sb + 1], scale=SCALE)
                if sb <= 1 and p == 0:
                    # bf16 shadow for queries [0, 256): strip0 psum cols
                    # [0,256) -> block cols [0,256); strip1 psum cols [0,128)
                    # (queries [128,256)) -> block cols [128,256)
                    q16 = 256 - 128 * par
                    nc.scalar.activation(
                        exb16_view()[:, :, par, 256 - q16:256],
                        srcv[:, :, 0:q16],
                        mybir.ActivationFunctionType.Exp,
                        bias=mb_t[:, sb:sb + 1], scale=SCALE)

            def tri_strip(h, sb):
                # causal mask on the diagonal 128x128 block of strip sb
                sp, par = divmod(sb, 2)
                pw0 = _sp_pieces(sp)[0][1]
                o = SP_OFF[sp] + (2 * (h % 2) + par) * pw0 + par * 128
                nc.vector.tensor_tensor(
                    exb[:, o:o + 128], exb[:, o:o + 128], tri_t[:, :],
                    op=mybir.AluOpType.mult)
                if sb <= 1:
                    o16 = (2 * (h % 2) + par) * 256 + par * 128
                    nc.vector.tensor_tensor(
                        exb16[:, o16:o16 + 128], exb16[:, o16:o16 + 128],
                        tri_t[:, :], op=mybir.AluOpType.mult)

            def pv_chunk(h, tc0, W=TCH):
                ps = b1_ps.tile([80, W], F32, tag="b1", name="pv",
                                padded_shape=[80, TCH])
                hh = h % 2
                if tc0 + W <= 256:
                    # bf16 path: strips 0 and 1 from the bf16 shadow
                    for par in range(2):
                        nc.tensor.matmul(
                            ps[0:D + 1, :],
                            vt16[:, h, par, :],
                            exb16_view()[:, hh, par, tc0:tc0 + W],
                            start=(par == 0), stop=(par == 1),
                        )
                    st = stage.tile([D + 1, W], BF16, tag="st", name="st",
                                    padded_shape=[D + 1, TCH])
                    nc.vector.tensor_copy(st[:, :], ps[0:D + 1, :])
                    out_eng = nc.sync if (h + tc0 // 128) % 2 == 0 else nc.gpsimd
                    out_eng.dma_start(out_d.ap()[h, :, tc0:tc0 + W], st[:, :])
                    return
                segs = []
                for sp in range(NSP):
                    g = 256 * sp
                    if g >= tc0 + W:
                        break
                    for rp, pw in _sp_pieces(sp):
                        s_lo = max(tc0, g + rp)
                        s_hi = min(tc0 + W, g + rp + pw)
                        if s_lo >= s_hi:
                            continue
                        # a window that never reaches the odd strip's keys
                        # uses a single-row matmul on the even strip only
                        # (the odd v tile may not be written yet)
                        segs.append((sp, rp, pw, s_lo - g - rp,
                                     s_lo - tc0, s_hi - s_lo,
                                     s_hi > g + 128))
                for i, (sp, rp, pw, c0, oc0, w, dr) in enumerate(segs):
                    view = exb_view(sp, rp, pw)
                    nc.tensor.matmul(
                        ps[:, oc0:oc0 + w],
                        vtp[sp][:, h, :, :] if dr else vtp[sp][:, h, 0, :],
                        view[:, hh, :, c0:c0 + w] if dr
                        else view[:, hh, 0, c0:c0 + w],
                        start=(i == 0), stop=(i == len(segs) - 1),
                        perf_mode=DR if dr else None,
                    )
                st = stage.tile([D + 1, W], BF16, tag="st", name="st",
                                padded_shape=[D + 1, TCH])
                nc.vector.tensor_copy(st[:, :], ps[0:D + 1, :])
                out_eng = nc.sync if (h + tc0 // 128) % 2 == 0 else nc.gpsimd
                out_eng.dma_start(out_d.ap()[h, :, tc0:tc0 + W], st[:, :])

            # pv windows: (emit-at-strip, tc0, width). The tail windows are
            # split so most of the last chunk's accumulation runs before the
            # final strips, shrinking the pair-boundary stall.
            PV_WINDOWS = {3: [(0, 256), (256, 256)], 7: [(512, 512)],
                          11: [(1024, 512)], 13: [(1536, 256)],
                          14: [(1792, 128)], 15: [(1920, 128)]}

            # ---- pipelined emission: a global slot pipeline ----
            # Pair p's strips occupy slots [12p, 12p+16); consecutive pairs
            # overlap by 4 slots so ACT never idles at pair boundaries. A
            # piece whose expT region is still to be read by the previous
            # pair's late pv windows (t >= 1536) is deferred past them.
            NSLOT = 41
            slot_work = [[] for _ in range(NSLOT)]   # items: (kind, fn)
            # kind 0: scores pieces/tri; 1: proj fillers; 2: pv windows

            # pair (0,1) cascaded start: emit chunk-0 projection chains
            # t-ascending; after each chain emit every scores piece whose q/k
            # columns are available, so the first exp fires early.
            all_pieces = [(256 * (sb // 2) + qb + qw, sb, p, rp, pw, qb, qw)
                          for sb in range(NSB)
                          for p, (rp, pw, qb, qw) in enumerate(_strip_pieces(sb))]
            all_pieces.sort(key=lambda x: (x[0], x[1]))
            emitted = set()
            tri_done = set()

            def emit_ready(limit):
                for need, sb, p, rp, pw, qb, qw in all_pieces:
                    if need > limit:
                        break
                    if (sb, p) in emitted:
                        continue
                    qk_exp_piece(0, sb, p, rp, pw, qb, qw)
                    emitted.add((sb, p))
                    if p == 0:
                        tri_strip(0, sb)
                        tri_strip(1, sb)
                        tri_done.add(sb)

            for t0 in range(0, T, 512):
                proj_qk_chain(0, t0, 0)
                proj_qk_chain(0, t0, 1)
                emit_ready(min(t0 + 512, 1024))

            def sched_pair(h0, base, defer_slot):
                for sb in range(NSB):
                    for p, (rp, pw, qb, qw) in enumerate(_strip_pieces(sb)):
                        if h0 == 0 and (sb, p) in emitted:
                            continue
                        t_end = 256 * (sb // 2) + qb + qw
                        sl = base + sb
                        if t_end > 1536 and sl < defer_slot:
                            sl = defer_slot
                        work = [(0, lambda h0=h0, sb=sb, p=p, rp=rp, pw=pw,
                                 qb=qb, qw=qw:
                                 qk_exp_piece(h0, sb, p, rp, pw, qb, qw))]
                        if p == 0 and not (h0 == 0 and sb in tri_done):
                            work.append((0, lambda h0=h0, sb=sb: tri_strip(h0, sb)))
                            work.append((0, lambda h0=h0, sb=sb: tri_strip(h0 + 1, sb)))
                        slot_work[sl].extend(work)
                    for tc0, W in PV_WINDOWS.get(sb, ()):
                        sl = max(base + sb, defer_slot if base else 0)
                        slot_work[sl].append(
                            (2, lambda h0=h0, tc0=tc0, W=W: pv_chunk(h0, tc0, W)))
                        slot_work[sl].append(
                            (2, lambda h0=h0, tc0=tc0, W=W: pv_chunk(h0 + 1, tc0, W)))

            sched_pair(0, 0, 0)
            sched_pair(2, 12, 16)
            sched_pair(4, 24, 28)
            # fillers: v projection paced over pair01's strips (1/slot, just
            # ahead of the pv windows); chunk-1/2 q/k projections early
            for tb in range(NSB):
                slot_work[max(tb, 2 + tb // 2) if tb < 4 else tb].append(
                    (1, lambda tb=tb: proj_v_chain(tb)))
            # c1/c2 chains placed as late as their consumers allow, to keep
            # the PE free for pair-0/1 scores and relax the weight-DMA
            # deadlines (c1 needed from slot 12, c2 from slot 24)
            for i in range(8):
                t0, wch = (i % 4) * 512, i // 4
                slot_work[4 + i].append(
                    (1, lambda t0=t0, w=wch: proj_qk_chain(1, t0, w)))
                slot_work[14 + i].append(
                    (1, lambda t0=t0, w=wch: proj_qk_chain(2, t0, w)))

            for sl in range(NSLOT):
                for _, f in sorted(slot_work[sl], key=lambda kf: kf[0]):
                    f()

    nc.compile()
    return nc


_NC_CACHE = None


def _get_nc():
    global _NC_CACHE
    if _NC_CACHE is None:
        _NC_CACHE = _build()
    return _NC_CACHE


def _pack_x(xb, dt=ml_dtypes.bfloat16):
    """[T, E] batch slice -> slice-major packed [128, ECH*T] (xT layout)."""
    xT = xb.T.reshape(ECH, 128, T // 512, 512)          # [e, p, s, t']
    return np.ascontiguousarray(
        xT.transpose(1, 2, 0, 3).reshape(128, ECH * T)).astype(dt)


def _pack_w(w_sl, dt=ml_dtypes.bfloat16, scale=1.0):
    """[384, 768] weight slice -> e-major packed [128, ECH*OC] (for wv:
    rhs slice for e-chunk at cols [OC*e, OC*(e+1)))."""
    wT = (w_sl.T * scale).reshape(ECH, 128, OC)         # [e, p, j]
    return np.ascontiguousarray(
        wT.transpose(1, 0, 2).reshape(128, ECH * OC)).astype(dt)


def _pack_w_cm(w_sl, dt=ml_dtypes.float8_e4m3, scale=W8SCALE):
    """[384, 768] weight slice -> chunk-major packed [128, ECH*OC]:
    lhsT for (chunk c, e-chunk e) at cols [768c+128e, 768c+128e+128)."""
    wT = (w_sl.T * scale).reshape(ECH, 128, QKC, 128)   # [e, p, c, j]
    return np.ascontiguousarray(
        wT.transpose(1, 2, 0, 3).reshape(128, ECH * OC)).astype(dt)


def kernel(hidden_states, attention_mask, Wq, bq, Wk, bk, Wv, bv):
    nc = _get_nc()
    in_maps = _make_in_maps(hidden_states, attention_mask, Wq, bq, Wk, bk, Wv, bv)
    res = bass_utils.run_bass_kernel_spmd(nc, in_maps, core_ids=list(range(NCORES)))
    return _assemble(res.results)


def _make_in_maps(hidden_states, attention_mask, Wq, bq, Wk, bk, Wv, bv):
    hidden_states = np.asarray(hidden_states, dtype=np.float32)
    attention_mask = np.asarray(attention_mask, dtype=np.float32)
    Wq, Wk, Wv = (np.asarray(w, dtype=np.float32) for w in (Wq, Wk, Wv))
    bq, bk, bv = (np.asarray(b, dtype=np.float32) for b in (bq, bk, bv))

    tri = np.tril(np.ones((128, 128), np.float32)).T.astype(ml_dtypes.bfloat16)
    # tri[s, t] = 1 where t >= s

    in_maps = []
    for c in range(NCORES):
        b, g = divmod(c, 2)
        sl = slice(OC * g, OC * (g + 1))
        m = attention_mask[b, 0, 0, :]
        in_maps.append({
            "xTp": _pack_x(hidden_states[b]),
            "xT8p": _pack_x(hidden_states[b], ml_dtypes.float8_e4m3),
            "wqp": _pack_w_cm(Wq[sl]),
            "wkp": _pack_w_cm(Wk[sl]),
            "wq16p": _pack_w_cm(Wq[sl], ml_dtypes.bfloat16, 1.0),
            "wk16p": _pack_w_cm(Wk[sl], ml_dtypes.bfloat16, 1.0),
            "wvp": _pack_w(Wv[sl]),
            "wv8p": _pack_w(Wv[sl], ml_dtypes.float8_e4m3, W8SCALE),
            "bq": np.ascontiguousarray(bq[sl]).reshape(QKC, 128, 1),
            "bk": np.ascontiguousarray(bk[sl]).reshape(QKC, 128, 1),
            "bvr": np.broadcast_to(bv[sl], (128, OC)).copy(),
            "mb": np.ascontiguousarray(m.reshape(NSB, 128).T),
            "tri": tri,
        })
    return in_maps


def _assemble(results):
    out = np.empty((B, T, E), np.float32)
    for c in range(NCORES):
        b, g = divmod(c, 2)
        oT = np.asarray(results[c]["outT"], dtype=np.float32)  # [6, 65, 2048]
        for h6 in range(HPC):
            h = HPC * g + h6
            out[b, :, D * h:D * h + D] = (oT[h6, :D] / oT[h6, D:D + 1]).T
    return out
